# revision 20
# baseline (speedup 1.0000x reference)
"""CapsuleNetwork (conv->BN->relu->primary caps->squash->dynamic routing) on 8 trn2 cores.

Strategy: pure data parallel over the flattened token axis N=B*S=8192 (1024
tokens/core).  Device kernel works in "tokens-on-free" layout: every on-chip
tensor is [feature-rows (<=128 partitions), token-columns].  All contractions
(GEMMs, W_route applications, partition-group reductions and broadcasts) run
on the PE array as fp32r matmuls; the per-token bilinear products (c*p and
p*rr) run on DVE/GPSIMD; transcendentals on ACT via the single
natural_log_exp table set (rsqrt x = exp(-0.5 ln x), 1/x = exp(-ln x)).

Layouts (per 512-token tile, tokens always on the free axis):
  xT, h, praw, p:   2 chunks [128, F], rows = feature (d / oc / (i,d))
  c, exp(blog), blog, a: 4 chunks [128, F], rows = (r, i), j = 4*chunk + r
  sv, v:            4 PSUM banks [128, F], j's 32-row slot = bank j//4,
                    rows 32*(j%4)+o, o<16 real, o>=16 zero-padded
  sq/ssv/Z scales:  [32|16, F] at partition base 0

Host-side (free) prep: x is passed pre-transposed per core, BN folded into
conv1, conv k=5 center taps pre-sliced, W_route pre-packed into matmul
operand layouts (incl. zero-padding + base-partition replication), and the
final (j,o)->(o,j) output permute + junk-row drop is numpy.
"""

import sys

sys.path.insert(0, "/opt/trn_rl_repo")

import numpy as np

import concourse.bacc as bacc
import concourse.mybir as mybir
from concourse import tile
from concourse.bass_utils import run_bass_kernel_spmd

B, S, D = 4, 2048, 256
PC, PD = 32, 8
OC, OD = 16, 16
BN_EPS = 1e-5
SQ_EPS = 1e-8
NCORES = 8
NTOK = B * S
NCORE_TOK = NTOK // NCORES  # 1024

F32 = mybir.dt.float32
F32R = mybir.dt.float32r
AF = mybir.ActivationFunctionType
ALU = mybir.AluOpType

# j's whose big per-token multiplies go via ACT-evac + GPSIMD instead of DVE.
GP_JS = frozenset(j for j in range(16) if j % 4 == 3)


def r32(ap):
    return ap.bitcast(F32R)


def host_prep(conv1_w, conv1_b, bn_gamma, bn_beta, bn_mean, bn_var, pc_w, pc_b, W_route):
    """Pack all weights into the exact SBUF layouts the device kernel uses."""
    f = np.float32
    scale = (bn_gamma / np.sqrt(bn_var + BN_EPS)).astype(f)
    w1_eff = conv1_w[:, :, 2].astype(f) * scale[:, None]  # [oc, d]
    w1t = np.ascontiguousarray(w1_eff.T)  # [d, oc]
    W1T = np.concatenate([w1t[:128], w1t[128:]], axis=1)  # [128, 512] cols=(kc,oc)
    b1 = ((conv1_b - bn_mean) * scale + bn_beta).astype(f)
    B1 = np.ascontiguousarray(b1.reshape(2, 128).T)  # [128, 2]

    w2t = np.ascontiguousarray(pc_w[:, :, 2].astype(f).T)  # [oc, (i,d)]
    W2T = np.concatenate([w2t[:128], w2t[128:]], axis=1)  # [128, 512]
    B2 = np.ascontiguousarray(pc_b.astype(f).reshape(2, 128).T)  # [128, 2]

    Wr = W_route.astype(f)  # [j, i, o, d]
    tt = Wr.transpose(1, 3, 0, 2)  # [i, d, j, o]
    flat = tt.reshape(256, 16, 16)  # [(i,d), j, o]

    # WSUM [128, 2kc * 4bank * 128]: sv1 = (W/16) @ p into the 4-bank sv layout
    # bank nb rows 32r+c: j=4nb+r, value W[j,i,c,d]/16 for c<16 else 0.
    WSUM = np.zeros((128, 2 * 4 * 128), f)
    for kc in range(2):
        for nb in range(4):
            blk = np.zeros((128, 128), f)
            for r in range(4):
                blk[:, 32 * r: 32 * r + 16] = flat[kc * 128:(kc + 1) * 128, 4 * nb + r] / 16.0
            WSUM[:, (kc * 4 + nb) * 128:(kc * 4 + nb) * 128 + 128] = blk

    # WSV [128, 16j * 2m * 128]: per (j, m) an [128, 128] lhsT whose only
    # nonzero cols are 32*(j%4)+o -- the 4 j's of one bank accumulate into a
    # full M=128 matmul at dst partition 0 (col-offset dsts are illegal).
    WSV = np.zeros((128, 4096), f)
    for j in range(16):
        for m in range(2):
            base = (j * 2 + m) * 128
            WSV[:, base + 32 * (j % 4): base + 32 * (j % 4) + 16] = \
                flat[m * 128:(m + 1) * 128, j]

    # WRR [128, 16j * 2m * 128]: rows 32q+o hold W[j,i,o,d] (replicated at
    # each 32-aligned base q so lhsT base matches the sv-slot rhs base).
    wrr = tt.transpose(3, 2, 0, 1).reshape(16, 16, 256)  # [o, j, (i,d)]
    WRR = np.zeros((128, 16 * 2 * 128), f)
    for q in range(4):
        for j in range(16):
            for m in range(2):
                WRR[32 * q: 32 * q + 16, j * 256 + m * 128: j * 256 + (m + 1) * 128] = \
                    wrr[:, j, m * 128:(m + 1) * 128]

    # EAD [128, 2m * 32]: chunk m reduces d-groups into cols 16m+i_rel.
    EAD = np.zeros((128, 64), f)
    for m in range(2):
        for p in range(128):
            EAD[p, m * 32 + 16 * m + p // 8] = 1.0
    # EADX [128, (m,r) * 128]: like EAD but shifted to cols 32r+16m+i_rel so a
    # whole blog chunk (4 j's) accumulates as M=128 matmuls at dst partition 0.
    EADX = np.zeros((128, 2 * 4 * 128), f)
    for m in range(2):
        for r in range(4):
            for p in range(128):
                EADX[p, (m * 4 + r) * 128 + 32 * r + 16 * m + p // 8] = 1.0
    # ESO [128, 4nb * 16]: bank nb: rows 32r+o (o<16) -> col j_local=4nb+r.
    ESO = np.zeros((128, 64), f)
    for nb in range(4):
        for r in range(4):
            for o in range(16):
                ESO[32 * r + o, nb * 16 + 4 * nb + r] = 1.0
    # EZ [128, 32]: rows (r,i) -> col i (softmax Z: sum over the chunk's 4 j's)
    EZ = np.zeros((128, 32), f)
    for p in range(128):
        EZ[p, p % 32] = 1.0
    # ERZ [32, 128]: row i -> cols (r,i)
    ERZ = np.ascontiguousarray(EZ.T)
    # ECX [128, 2m * 128]: lhsT slice [32 @ base 32r, 128] replicated at each
    # 32-base: E[i, (i_rel,d)] = delta(i, 16m+i_rel)
    ECX = np.zeros((128, 256), f)
    for r in range(4):
        for m in range(2):
            for p in range(128):
                ECX[32 * r + (16 * m + p // 8), m * 128 + p] = 1.0
    # ESV [16, 4q * 128]: row j -> cols (r,i) of chunk q where j=4q+r
    ESV = np.zeros((16, 512), f)
    for q in range(4):
        for p in range(128):
            ESV[4 * q + p // 32, q * 128 + p] = 1.0
    # EVO [16, 4nb * 128]: row j -> bank-nb cols 32r+o (o<16), j=4nb+r
    EVO = np.zeros((16, 512), f)
    for nb in range(4):
        for r in range(4):
            for o in range(16):
                EVO[4 * nb + r, nb * 128 + 32 * r + o] = 1.0

    out = dict(W1T=W1T, B1=B1, W2T=W2T, B2=B2, WSUM=WSUM, WSV=WSV, WRR=WRR,
               EAD=EAD, EADX=EADX, ESO=ESO, EZ=EZ, ERZ=ERZ, ECX=ECX, ESV=ESV,
               EVO=EVO, EPSB=np.full((128, 1), SQ_EPS, f), ONEB=np.full((128, 1), 1.0, f))
    return {k: np.ascontiguousarray(v.astype(f)) for k, v in out.items()}


WSHAPES = dict(
    W1T=[128, 512], B1=[128, 2], W2T=[128, 512], B2=[128, 2],
    WSUM=[128, 1024], WSV=[128, 4096], WRR=[128, 4096],
    EAD=[128, 64], EADX=[128, 1024], ESO=[128, 64], EZ=[128, 32], ERZ=[32, 128],
    ECX=[128, 256], ESV=[16, 512], EVO=[16, 512], EPSB=[128, 1], ONEB=[128, 1],
)


def build_module(n_core=NCORE_TOK, F=512, reps=1, gp_js=None, stages='full'):
    """Build the per-core Bass module.  Same NEFF on all 8 cores (SPMD)."""
    NT = n_core // F
    assert NT * F == n_core
    gp_set = GP_JS if gp_js is None else frozenset(gp_js)
    nc = bacc.Bacc("TRN2", target_bir_lowering=False, debug=False, num_devices=NCORES)

    xt_d = nc.dram_tensor("xt", [256, n_core], F32R, kind="ExternalInput")
    out_d = nc.dram_tensor("out", [512, n_core], F32, kind="ExternalOutput")
    wd = {k: nc.dram_tensor(k, shp, F32R, kind="ExternalInput")
          for k, shp in WSHAPES.items()}

    with tile.TileContext(nc) as tc:
        with (
            tc.tile_pool(name="wpool", bufs=1) as wpool,
            tc.tile_pool(name="xpool", bufs=2) as xpool,
            tc.tile_pool(name="hpool", bufs=2) as hpool,
            tc.tile_pool(name="ppool", bufs=2) as ppool,
            tc.tile_pool(name="cpool", bufs=2) as cpool,
            tc.tile_pool(name="qpool", bufs=2) as qpool,
            tc.tile_pool(name="blogpool", bufs=2) as blogpool,
            tc.tile_pool(name="smpool", bufs=1) as smpool,
            tc.tile_pool(name="vpool", bufs=2) as vpool,
            tc.tile_pool(name="ps_A", bufs=(2 if F >= 512 else 4), space="PSUM") as ps_A,
        ):
            w = {}
            for k, shp in WSHAPES.items():
                w[k] = wpool.tile(shp, F32, tag=f"w_{k}", name=f"w_{k}")
                nc.sync.dma_start(r32(w[k][:]), wd[k][:])

            F2, F4 = 2 * F, 4 * F

            def mm(out_ap, lhsT_ap, rhs_ap, start=True, stop=True, tp=(0, 0)):
                nc.tensor.matmul(out_ap, r32(lhsT_ap), r32(rhs_ap), start=start,
                                 stop=stop, tile_position=tp)

            def g3(ap):
                return ap.rearrange("p (g f) -> p g f", g=4)

            def b4(ap):
                # [128, F] -> [128, 4, F] with step-0 broadcast on the group dim
                return ap.unsqueeze(1).to_broadcast((128, 4, F))

            I32 = mybir.dt.int32
            MAGIC = 0x5F3759DF

            def dve_rsqrt(y, x, sc1, sc2, P_act, Fw):
                """y = 1/sqrt(x) entirely on DVE (bit-hack seed + 2 Newton steps).
                sc1/sc2: scratch tiles.  All APs [P_act, Fw] fp32 SBUF."""
                nc.vector.tensor_scalar(sc1.bitcast(I32), x.bitcast(I32), 1, None,
                                        op0=ALU.logical_shift_right)
                nc.vector.tensor_scalar(sc2.bitcast(I32), sc1.bitcast(I32), -1, None,
                                        op0=ALU.bitwise_xor)
                nc.vector.tensor_scalar(y.bitcast(I32), sc2.bitcast(I32), MAGIC + 1,
                                        None, op0=ALU.add)
                for _ in range(2):
                    nc.vector.tensor_tensor(sc1, y, y, ALU.mult)
                    nc.vector.tensor_tensor(sc2, sc1, x, ALU.mult)
                    nc.vector.tensor_scalar(sc1, sc2, -0.5, 1.5, op0=ALU.mult,
                                            op1=ALU.add)
                    nc.vector.tensor_tensor(y, y, sc1, ALU.mult)

            def squash_scale(dst, sq_ap, P_act, Fw):
                """dst = sqrt(sq+eps)/(1+sq) = sq/(1+sq)/sqrt(sq+eps)·(1+eps/sq);
                the eps/sq relative error is bounded by eps=1e-8.  Computed in
                ln-space on ACT (2 Ln + 1 Exp, same natural_log_exp table) with
                one DVE combine: exp(0.5·ln(sq+eps) − ln(1+sq))."""
                l1 = smpool.tile([128, F], F32, tag="sq_l1", name="sq_l1")
                l2 = smpool.tile([128, F], F32, tag="sq_l2", name="sq_l2")
                u = smpool.tile([128, F], F32, tag="sq_u", name="sq_u")
                a = (slice(0, P_act), slice(0, Fw))
                nc.scalar.activation(r32(l1[a]), sq_ap, AF.Ln, bias=w["EPSB"][:P_act, :])
                nc.scalar.activation(r32(l2[a]), sq_ap, AF.Ln, bias=w["ONEB"][:P_act, :])
                nc.vector.scalar_tensor_tensor(u[a], l1[a], 0.5, l2[a],
                                               op0=ALU.mult, op1=ALU.subtract)
                nc.scalar.activation(dst, u[a], AF.Exp)

            def phase_front(st, t_i):
                """DMA in + GEMM1 + GEMM2 + squash(p); leaves P2/BLOG4 in st."""
                cols = st["cols"]
                XT2 = xpool.tile([128, F2], F32, tag="xt2", name="xt2")
                for m in range(2):
                    nc.sync.dma_start(r32(XT2[:, m * F:(m + 1) * F]),
                                      xt_d[m * 128:(m + 1) * 128, cols])
                H2 = hpool.tile([128, F2], F32, tag="h2", name="h2")
                pg = ps_A.tile([128, F2], F32, tag="A", name="pg1")
                for mc in range(2):
                    for kc in range(2):
                        mm(pg[:, mc * F:(mc + 1) * F],
                           w["W1T"][:, kc * 256 + mc * 128: kc * 256 + mc * 128 + 128],
                           XT2[:, kc * F:(kc + 1) * F], start=(kc == 0), stop=(kc == 1))
                for mc in range(2):
                    nc.scalar.activation(r32(H2[:, mc * F:(mc + 1) * F]),
                                         pg[:, mc * F:(mc + 1) * F], AF.Relu,
                                         bias=w["B1"][:, mc:mc + 1])
                PRAW2 = ppool.tile([128, F2], F32, tag="praw2", name="praw2")
                pg2 = ps_A.tile([128, F2], F32, tag="A", name="pg2")
                for mc in range(2):
                    for kc in range(2):
                        mm(pg2[:, mc * F:(mc + 1) * F],
                           w["W2T"][:, kc * 256 + mc * 128: kc * 256 + mc * 128 + 128],
                           H2[:, kc * F:(kc + 1) * F], start=(kc == 0), stop=(kc == 1))
                    nc.scalar.activation(PRAW2[:, mc * F:(mc + 1) * F],
                                         pg2[:, mc * F:(mc + 1) * F], AF.Identity,
                                         bias=w["B2"][:, mc:mc + 1])
                SQT2 = qpool.tile([128, F2], F32, tag="q4a", name="sqt2")
                nc.scalar.activation(r32(SQT2[:]), PRAW2[:], AF.Square)
                pq = ps_A.tile([128, F], F32, tag="svg", bufs=4, name="pq")
                for m in range(2):
                    mm(pq[0:32, :F], w["EAD"][:, m * 32:(m + 1) * 32],
                       SQT2[:, m * F:(m + 1) * F], start=(m == 0), stop=(m == 1))
                SP = smpool.tile([128, F], F32, tag="s_p", name="s_p")
                squash_scale(r32(SP[:32, :F]), pq[:32, :F], 32, F)
                psx = ps_A.tile([128, F2], F32, tag="A", name="psx")
                for m in range(2):
                    mm(psx[:, m * F:(m + 1) * F], w["ECX"][:32, m * 128:(m + 1) * 128],
                       SP[:32, :F])
                P2 = ppool.tile([128, F2], F32, tag="p2", name="p2")
                nc.vector.scalar_tensor_tensor(
                    r32(P2[:]), psx[:, :F2], 1.0, PRAW2[:], op0=ALU.mult, op1=ALU.mult)
                st["P2"] = P2
                st["BLOG4"] = blogpool.tile([128, F4], F32, tag="blog4", name="blog4")

            def phase_iter(st, t_i, it):
                P2, BLOG4, cols = st["P2"], st["BLOG4"], st["cols"]
                SVS4 = vpool.tile([128, F4], F32, tag="svs4", name="svs4")
                if it == 1:
                    for g in range(4):
                        svg = ps_A.tile([128, F], F32, tag="svg", bufs=4, name="svg")
                        for kc in range(2):
                            mm(svg[:, :F],
                               w["WSUM"][:, (kc * 4 + g) * 128:(kc * 4 + g) * 128 + 128],
                               P2[:, kc * F:(kc + 1) * F],
                               start=(kc == 0), stop=(kc == 1))
                        nc.scalar.activation(r32(SVS4[:, g * F:(g + 1) * F]), svg[:, :F],
                                             AF.Copy)
                else:
                    # softmax over j
                    EB4 = cpool.tile([128, F4], F32, tag="eb4", name="eb4")
                    nc.scalar.activation(r32(EB4[:]), BLOG4[:], AF.Exp)
                    pzx = ps_A.tile([128, F2], F32, tag="A", name="pzx")
                    for q in range(4):
                        mm(pzx[:32, :F], w["EZ"][:], EB4[:, q * F:(q + 1) * F],
                           start=(q == 0), stop=(q == 3))
                    RZ = smpool.tile([128, F], F32, tag="rz", name="rz")
                    with nc.allow_low_precision("f32r round of 1/Z"):
                        nc.vector.reciprocal(r32(RZ[:32, :F]), pzx[:32, :F])
                    przx = ps_A.tile([128, F2], F32, tag="A", name="przx")
                    for m in range(2):
                        mm(przx[:, m * F:(m + 1) * F],
                           w["ECX"][:32, m * 128:(m + 1) * 128], RZ[:32, :F])
                    PZ2 = cpool.tile([128, F2], F32, tag="c4", name="pz2")
                    nc.vector.scalar_tensor_tensor(
                        r32(PZ2[:]), przx[:, :F2], 1.0, P2[:], op0=ALU.mult,
                        op1=ALU.mult)
                    # q = cexp * p ; sv = WSV^T q   (4 j's per group g)
                    for g in range(4):
                        Q4 = [None, None]
                        for m in range(2):
                            Q4[m] = qpool.tile([128, F4], F32, tag=f"q4{'ab'[m]}",
                                               name=f"q4{'ab'[m]}")
                            for h in range(2):
                                cx2 = ps_A.tile([128, F2], F32, tag="A", name="cx2")
                                for rh in range(2):
                                    r = 2 * h + rh
                                    mm(cx2[:, rh * F:(rh + 1) * F],
                                       w["ECX"][r * 32:(r + 1) * 32, m * 128:(m + 1) * 128],
                                       EB4[r * 32:(r + 1) * 32, g * F:(g + 1) * F],
                                       tp=(r * 32, 0))
                                nc.vector.scalar_tensor_tensor(
                                    r32(Q4[m][:, h * F2:(h + 1) * F2]).rearrange(
                                        "p (g f) -> p g f", g=2),
                                    cx2[:].rearrange("p (g f) -> p g f", g=2), 1.0,
                                    PZ2[:, m * F:(m + 1) * F].unsqueeze(1).to_broadcast(
                                        (128, 2, F)),
                                    op0=ALU.mult, op1=ALU.mult)
                        svg = ps_A.tile([128, F], F32, tag="svg", bufs=4, name="svg")
                        for m in range(2):
                            for r in range(4):
                                j = 4 * g + r
                                mm(svg[:, :F],
                                   w["WSV"][:, (j * 2 + m) * 128:(j * 2 + m + 1) * 128],
                                   Q4[m][:, r * F:(r + 1) * F],
                                   start=(m == 0 and r == 0),
                                   stop=(m == 1 and r == 3))
                        nc.scalar.activation(r32(SVS4[:, g * F:(g + 1) * F]), svg[:, :F],
                                             AF.Copy)

                # ---- squash(sv) ----
                SQ24 = qpool.tile([128, F4], F32, tag="q4a", name="sq24")
                nc.scalar.activation(r32(SQ24[:]), SVS4[:], AF.Square)
                pq2 = ps_A.tile([128, F], F32, tag="svg", bufs=4, name="pq2")
                for g in range(4):
                    mm(pq2[0:16, :F], w["ESO"][:, g * 16:(g + 1) * 16],
                       SQ24[:, g * F:(g + 1) * F], start=(g == 0), stop=(g == 3))
                SSV = smpool.tile([128, F], F32, tag="ssv", name="ssv")
                squash_scale(r32(SSV[:16, :F]), pq2[:16, :F], 16, F)

                if it < 3:
                    SE4 = smpool.tile([128, F4], F32, tag="se4", name="se4")
                    for q in range(4):
                        pseq = ps_A.tile([128, F], F32, tag="svg", bufs=4, name="pseq")
                        mm(pseq[:, :F], w["ESV"][:, q * 128:(q + 1) * 128], SSV[:16, :F])
                        nc.scalar.activation(SE4[:, q * F:(q + 1) * F], pseq[:, :F],
                                             AF.Copy)
                    for g in range(4):
                        pag = ps_A.tile([128, F], F32, tag="svg", bufs=4, name="pag")
                        for m in range(2):
                            rrh = [None, None]
                            for h in range(2):
                                rr2 = ps_A.tile([128, F2], F32, tag="A", name="rr2")
                                rrh[h] = rr2
                                for rh in range(2):
                                    r = 2 * h + rh
                                    j = 4 * g + r
                                    sl = 32 * r
                                    mm(rr2[:, rh * F:(rh + 1) * F],
                                       w["WRR"][sl:sl + 16,
                                                j * 256 + m * 128: j * 256 + (m + 1) * 128],
                                       SVS4[sl:sl + 16, g * F:(g + 1) * F], tp=(sl, 0))
                            PR4 = qpool.tile([128, F4], F32, tag=f"q4{'ab'[m]}",
                                             name=f"pr4{'ab'[m]}")
                            for h in range(2):
                                nc.vector.scalar_tensor_tensor(
                                    r32(PR4[:, h * F2:(h + 1) * F2]).rearrange(
                                        "p (g f) -> p g f", g=2),
                                    rrh[h][:].rearrange("p (g f) -> p g f", g=2), 1.0,
                                    P2[:, m * F:(m + 1) * F].unsqueeze(1).to_broadcast(
                                        (128, 2, F)),
                                    op0=ALU.mult, op1=ALU.mult)
                            for r in range(4):
                                mm(pag[:, :F],
                                   w["EADX"][:, (m * 4 + r) * 128:(m * 4 + r + 1) * 128],
                                   PR4[:, r * F:(r + 1) * F],
                                   start=(m == 0 and r == 0),
                                   stop=(m == 1 and r == 3))
                        if it == 1:
                            nc.vector.scalar_tensor_tensor(
                                BLOG4[:, g * F:(g + 1) * F], pag[:, :F], 1.0,
                                SE4[:, g * F:(g + 1) * F],
                                op0=ALU.mult, op1=ALU.mult)
                        else:
                            TAg = smpool.tile([128, F], F32, tag="ta4", name="tag")
                            nc.vector.scalar_tensor_tensor(
                                TAg[:, :F], pag[:, :F], 1.0,
                                SE4[:, g * F:(g + 1) * F],
                                op0=ALU.mult, op1=ALU.mult)
                            nc.vector.tensor_tensor(BLOG4[:, g * F:(g + 1) * F],
                                                    BLOG4[:, g * F:(g + 1) * F],
                                                    TAg[:, :F], ALU.add)
                else:
                    SEV4 = smpool.tile([128, F4], F32, tag="se4", name="sev4")
                    for g in range(4):
                        pvq = ps_A.tile([128, F], F32, tag="svg", bufs=4, name="pvq")
                        mm(pvq[:, :F], w["EVO"][:, g * 128:(g + 1) * 128], SSV[:16, :F])
                        nc.scalar.activation(SEV4[:, g * F:(g + 1) * F], pvq[:, :F],
                                             AF.Copy)
                    V4 = qpool.tile([128, F4], F32, tag="q4b", name="v4")
                    nc.vector.scalar_tensor_tensor(
                        V4[:], SVS4[:], 1.0, SEV4[:], op0=ALU.mult, op1=ALU.mult)
                    for g in range(4):
                        nc.sync.dma_start(out_d[g * 128:(g + 1) * 128, cols],
                                          V4[:, g * F:(g + 1) * F])

            # Unroll 2 reps per hw-loop iteration when possible: the For_i
            # all-engine barrier only fires between iterations, so the second
            # rep's GEMM front overlaps the first rep's routing tail.
            unroll = 2 if reps % 2 == 0 else 1
            with tc.For_i(0, reps // unroll) as _rep_iv:
                for _u in range(unroll):
                    state = [{"cols": slice(t * F, (t + 1) * F)} for t in range(NT)]
                    for t_i in range(NT):
                        phase_front(state[t_i], t_i)
                    for it in (1, 2, 3):
                        for t_i in range(NT):
                            phase_iter(state[t_i], t_i, it)

    _dedupe_act_table_loads(nc)
    nc.finalize()
    return nc


def _dedupe_act_table_loads(nc):
    """All AFs used here (Relu/Identity/Square/Copy/Ln/Exp) live in the single
    natural_log_exp_and_others table set, but the placement pass picks a
    first-containing set per activation and ends up ping-ponging table loads
    inside the loop body.  Override: let the pass run, then replace its loads
    with one entry-block load of the covering set."""
    from concourse.hw_specs import get_activation_tables

    orig = nc.insert_act_table_loads

    def patched():
        orig()
        tables = list(get_activation_tables(nc.m.arch).items())
        target = next(i for i, (name, _) in enumerate(tables)
                      if name == "natural_log_exp_and_others")
        tset = tables[target][1]
        funcs = {ins.func for b in nc.main_func.blocks for ins in b.instructions
                 if isinstance(ins, mybir.InstActivation)}
        assert funcs <= tset, f"AFs outside natural_log_exp set: {funcs - tset}"
        first_load = None
        for b in nc.main_func.blocks:
            keep = []
            for ins in b.instructions:
                if isinstance(ins, mybir.InstLoadActFuncSet):
                    si = ins.sync_info
                    has_sync = si is not None and (
                        len(si.on_wait) > 0 or len(si.on_update) > 0)
                    if has_sync or first_load is None:
                        ins.act_func_set_id = target
                        if first_load is None:
                            first_load = ins
                        keep.append(ins)
                    continue
                keep.append(ins)
            b.set_instructions_from_list(keep) if hasattr(b, "set_instructions_from_list") else None
            if not hasattr(b, "set_instructions_from_list"):
                del b.instructions[:]
                for ins in keep:
                    b.instructions.append(ins)

    nc.insert_act_table_loads = patched


_NC_CACHE = {}


def _get_nc(n_core, F, reps=1):
    key = (n_core, F, reps)
    if key not in _NC_CACHE:
        _NC_CACHE[key] = build_module(n_core, F, reps)
    return _NC_CACHE[key]


def unpack_out(out, n):
    """device out [512, n] -> v [n, j, o]"""
    v = out.reshape(4, 4, 32, n)[:, :, :16, :]  # [nb, r, o, n]
    v = v.transpose(3, 0, 1, 2).reshape(n, 16, 16)  # [n, j=4nb+r, o]
    return v


def kernel(x, conv1_w, conv1_b, bn_gamma, bn_beta, bn_mean, bn_var, pc_w, pc_b, W_route):
    x = np.asarray(x, np.float32)
    weights = host_prep(
        np.asarray(conv1_w), np.asarray(conv1_b), np.asarray(bn_gamma),
        np.asarray(bn_beta), np.asarray(bn_mean), np.asarray(bn_var),
        np.asarray(pc_w), np.asarray(pc_b), np.asarray(W_route))

    xt = np.ascontiguousarray(x.reshape(NTOK, D).T)  # [256, 8192]
    nc = _get_nc(NCORE_TOK, 512)
    in_maps = []
    for c in range(NCORES):
        m = {"xt": np.ascontiguousarray(xt[:, c * NCORE_TOK:(c + 1) * NCORE_TOK])}
        m.update(weights)
        in_maps.append(m)
    res = run_bass_kernel_spmd(nc, in_maps, list(range(NCORES)))
    v = np.concatenate(
        [unpack_out(res.results[c]["out"], NCORE_TOK) for c in range(NCORES)], axis=0)
    out = np.swapaxes(v, 1, 2).reshape(B, S, OD * OC)  # [n, o, j] flattened
    return np.ascontiguousarray(out.astype(np.float32))

